# revision 4
# baseline (speedup 1.0000x reference)
"""Trainium2 Bass kernel for nn_DSA (dual-stage attention RNN).

Mathematical collapse used (exact, not approximate):
  - In the reference scan, beta = log_softmax(sc, axis=-1) over a SIZE-1
    axis, which is identically zero for any finite input.  Hence
    ctx_new = einsum('bt,bth->bh', 0, enc_h) == 0 exactly, so the carried
    context is zero at every step and the decoder input at step t is
    din_t = d[:, t] * dec_w[0,0] + dec_b[0].
  - The carried h_s is never read inside the step, so only the final
    step's h_s (t = T-2) reaches the head.  The encoder LSTM, s1, and the
    whole attention pipeline are dead code w.r.t. the output.
  - feat = [h_s, ctx] with ctx == 0, so the head reduces to
      out[b] = h_s[b,:] @ v + k0,
      v  = d1_w[:, :H].T @ d2_w[0,:],     k0 = d1_b @ d2_w[0,:] + d2_b[0]
  where h_s = sigmoid(o) * tanh(sigmoid(i) * tanh(g)) and
  [i,f,g,o] = din * W_ih_d[:,0] + b_d  (f unused since c0 == 0).

Sharding: pure data parallel over batch (B=32 -> 4 rows per core x 8).
All weights replicated; each core computes its 4 outputs independently.
"""

import numpy as np

import concourse.bacc as bacc
import concourse.bass as bass
import concourse.mybir as mybir
import concourse.tile as tile
from concourse import bass_utils

N_CORES = 8
B, T, H, L = 32, 100, 128, 64
BS = B // N_CORES  # batch rows per core

F32 = mybir.dt.float32
AF = mybir.ActivationFunctionType
ALU = mybir.AluOpType

_BUILD_CACHE = {}


def _build_nc():
    """Trace + compile the per-core Bass program (shapes are static)."""
    nc = bacc.Bacc("TRN2", target_bir_lowering=False, debug=False)

    d_in = nc.dram_tensor("d_s", (BS, T - 1), F32, kind="ExternalInput")
    wihd = nc.dram_tensor("w_ihd", (4 * H, 1), F32, kind="ExternalInput")
    b_d = nc.dram_tensor("b_d", (4 * H, 1), F32, kind="ExternalInput")
    decw = nc.dram_tensor("dec_w", (1, H + 1), F32, kind="ExternalInput")
    decb = nc.dram_tensor("dec_b", (1, 1), F32, kind="ExternalInput")
    d1w = nc.dram_tensor("d1_w", (H, 2 * H), F32, kind="ExternalInput")
    d1b = nc.dram_tensor("d1_b", (1, H), F32, kind="ExternalInput")
    d2w = nc.dram_tensor("d2_w", (1, H), F32, kind="ExternalInput")
    d2b = nc.dram_tensor("d2_b", (1, 1), F32, kind="ExternalInput")
    out = nc.dram_tensor("out", (BS, 1), F32, kind="ExternalOutput")

    with tile.TileContext(nc) as tc:
        with (
            tc.tile_pool(name="sb", bufs=1) as sb,
            tc.tile_pool(name="ps", bufs=1, space=bass.MemorySpace.PSUM) as ps,
        ):
            # --- input tiles -------------------------------------------------
            lhsT = sb.tile([2, 3 * H], F32)  # row0 = W chunks (i|g|o), row1 = b_d chunks
            rhs = sb.tile([2, BS], F32)      # row0 = din, row1 = ones
            dsb = sb.tile([1, BS], F32)      # d[:, T-2] as a row
            dwt = sb.tile([1, 1], F32)       # dec_w[0,0]
            dbt = sb.tile([1, 1], F32)       # dec_b[0]
            d1wt = sb.tile([H, H], F32)      # d1_w[:, :H]   layout [j, h]
            d2col = sb.tile([H, 1], F32)     # d2_w as a column
            d2wr = sb.tile([1, H], F32)      # d2_w as a row
            d1br = sb.tile([1, H], F32)      # d1_b as a row
            d2bt = sb.tile([1, 1], F32)      # d2_b[0]

            dma = nc.sync.dma_start
            dma(dsb[:, :], d_in[:, T - 2:T - 1].rearrange("a b -> b a"))
            dma(dwt[:, :], decw[0:1, 0:1])
            dma(dbt[:, :], decb[:, :])
            # gate order in z: i f g o ; f is dead (c0 == 0), so pack i|g|o
            dma(lhsT[0:1, 0:H], wihd[0:H, 0:1].rearrange("a b -> b a"))
            dma(lhsT[0:1, H:3 * H], wihd[2 * H:4 * H, 0:1].rearrange("a b -> b a"))
            dma(lhsT[1:2, 0:H], b_d[0:H, 0:1].rearrange("a b -> b a"))
            dma(lhsT[1:2, H:3 * H], b_d[2 * H:4 * H, 0:1].rearrange("a b -> b a"))
            dma(d1wt[:, :], d1w[:, 0:H])
            dma(d2col[:, :], d2w[0:1, :].rearrange("a b -> b a"))
            dma(d2wr[:, :], d2w[:, :])
            dma(d1br[:, :], d1b[:, :])
            dma(d2bt[:, :], d2b[:, :])

            # --- decoder input din = d*dec_w00 + dec_b0 ----------------------
            onesr = sb.tile([1, BS], F32)  # matmul lhsT must start at partition 0
            nc.vector.memset(onesr[:, :], 1.0)
            # engines can only address base partition 0/32/64: fill both rows
            # with 1.0, then overwrite row 0 with din
            nc.vector.memset(rhs[:, :], 1.0)
            nc.vector.tensor_scalar(
                rhs[0:1, :], dsb[:, :], dwt[:, :], dbt[:, :], ALU.mult, ALU.add
            )

            # --- LSTM gates: z[p, g*BS+b] = W_g[p]*din[b] + b_g[p] -----------
            zps = ps.tile([H, 3 * BS], F32)
            for gi in range(3):
                nc.tensor.matmul(
                    zps[:, gi * BS:(gi + 1) * BS],
                    lhsT[:, gi * H:(gi + 1) * H],
                    rhs[:, :],
                    start=True,
                    stop=True,
                )

            si = sb.tile([H, BS], F32)
            so = sb.tile([H, BS], F32)
            tg = sb.tile([H, BS], F32)
            nc.scalar.activation(si[:, :], zps[:, 0:BS], AF.Sigmoid)
            nc.scalar.activation(so[:, :], zps[:, 2 * BS:3 * BS], AF.Sigmoid)
            nc.scalar.activation(tg[:, :], zps[:, BS:2 * BS], AF.Tanh)
            cst = sb.tile([H, BS], F32)
            nc.vector.tensor_mul(cst[:, :], si[:, :], tg[:, :])
            tcs = sb.tile([H, BS], F32)
            nc.scalar.activation(tcs[:, :], cst[:, :], AF.Tanh)
            hst = sb.tile([H, BS], F32)  # h_s with hidden dim on partitions
            nc.vector.tensor_mul(hst[:, :], so[:, :], tcs[:, :])

            # --- head: v[h] = sum_j d1_w[j,h]*d2_w[j]; k0 = d1_b.d2_w + d2_b -
            vps = ps.tile([H, 1], F32)
            nc.tensor.matmul(vps[:, :], d1wt[:, :], d2col[:, :], start=True, stop=True)
            vcol = sb.tile([H, 1], F32)
            nc.vector.tensor_copy(vcol[:, :], vps[:, :])

            prod = sb.tile([1, H], F32)
            nc.vector.tensor_mul(prod[:, :], d1br[:, :], d2wr[:, :])
            k0t = sb.tile([1, 1], F32)
            nc.vector.tensor_reduce(k0t[:, :], prod[:, :], mybir.AxisListType.X, ALU.add)
            nc.vector.tensor_scalar_add(k0t[:, :], k0t[:, :], d2bt[:, :])

            # --- out[b] = sum_h h[h,b]*v[h] + k0 -----------------------------
            ops = ps.tile([BS, 1], F32)
            nc.tensor.matmul(ops[:, :], hst[:, :], vcol[:, :], start=True, stop=False)
            nc.tensor.matmul(ops[:, :], onesr[:, :], k0t[:, :], start=False, stop=True)

            osb = sb.tile([BS, 1], F32)
            nc.vector.tensor_copy(osb[:, :], ops[:, :])
            dma(out[:, :], osb[:, :])

    nc.compile()
    return nc


def get_nc():
    if "nc" not in _BUILD_CACHE:
        _BUILD_CACHE["nc"] = _build_nc()
    return _BUILD_CACHE["nc"]


def make_in_maps(inputs):
    f = lambda k: np.ascontiguousarray(np.asarray(inputs[k]), dtype=np.float32)
    d = f("d")
    wihd = f("W_ih_d").reshape(4 * H, 1)
    b_d = f("b_d").reshape(4 * H, 1)
    decw = f("dec_w").reshape(1, H + 1)
    decb = f("dec_b").reshape(1, 1)
    d1w = f("d1_w").reshape(H, 2 * H)
    d1b = f("d1_b").reshape(1, H)
    d2w = f("d2_w").reshape(1, H)
    d2b = f("d2_b").reshape(1, 1)
    in_maps = []
    for c in range(N_CORES):
        in_maps.append(
            {
                "d_s": np.ascontiguousarray(d[c * BS:(c + 1) * BS]),
                "w_ihd": wihd,
                "b_d": b_d,
                "dec_w": decw,
                "dec_b": decb,
                "d1_w": d1w,
                "d1_b": d1b,
                "d2_w": d2w,
                "d2_b": d2b,
            }
        )
    return in_maps


def run_spmd(inputs, trace=False):
    """Returns (full_output (B,), BassKernelResults)."""
    nc = get_nc()
    res = bass_utils.run_bass_kernel_spmd(
        nc, make_in_maps(inputs), list(range(N_CORES)), trace=trace
    )
    outs = [np.asarray(res.results[c]["out"]).reshape(BS) for c in range(N_CORES)]
    full = np.concatenate(outs).astype(np.float32)
    return full, res


def kernel(**inputs) -> np.ndarray:
    full, _ = run_spmd(inputs, trace=False)
    return full


# revision 7
# speedup vs baseline: 1.1776x; 1.1776x over previous
"""Trainium2 Bass kernel for nn_DSA (dual-stage attention RNN).

Mathematical collapse used (exact, not approximate):
  - In the reference scan, beta = log_softmax(sc, axis=-1) over a SIZE-1
    axis, which is identically zero for any finite input.  Hence
    ctx_new = einsum('bt,bth->bh', 0, enc_h) == 0 exactly, so the carried
    context is zero at every step and the decoder input at step t is
    din_t = d[:, t] * dec_w[0,0] + dec_b[0].
  - The carried h_s is never read inside the step, so only the final
    step's h_s (t = T-2) reaches the head.  The encoder LSTM, s1, and the
    whole attention pipeline are dead code w.r.t. the output.
  - feat = [h_s, ctx] with ctx == 0, so the head reduces to
      out[b] = h_s[b,:] @ v + k0,
      v  = d1_w[:, :H].T @ d2_w[0,:],     k0 = d1_b @ d2_w[0,:] + d2_b[0]
  where h_s = sigmoid(o) * tanh(sigmoid(i) * tanh(g)) and
  [i,f,g,o] = din * W_ih_d[:,0] + b_d  (f unused since c0 == 0).

Sharding: pure data parallel over batch (B=32 -> 4 rows per core x 8).
All weights replicated; each core computes its 4 outputs independently.

Device layout (per core, BS=4):
  - batch on partitions.  z (BS, 384) = (Wrep * din) + brep in ONE DVE
    scalar_tensor_tensor, where Wrep/brep are the replicated weight rows
    (pure host-side layout, no host arithmetic).  Gate order packed as
    [i|o|g] so both sigmoids run as a single ACT op on a (BS,256) slice.
  - v is computed on PE directly in replicated form:
    vrep (BS,128) = (d2w_col x4).T @ d1w   (one K=128 matmul, no copies)
    krep (BS,1)   = (d2w_col x4).T @ d1b_col  (+ ones.T @ d2b accumulate)
  - final: tensor_tensor_reduce(h * vrep, add-reduce, init=krep) gives
    out (BS,1) in one DVE op.
"""

import numpy as np

import concourse.bacc as bacc
import concourse.bass as bass
import concourse.mybir as mybir
import concourse.tile as tile
from concourse import bass_utils

N_CORES = 8
B, T, H, L = 32, 100, 128, 64
BS = B // N_CORES  # batch rows per core

F32 = mybir.dt.float32
AF = mybir.ActivationFunctionType
ALU = mybir.AluOpType

PC_COLS = 6 * H + 3   # packC: [W_i|W_o|W_g | b_i|b_o|b_g | dw | db | d2b]
PB_COLS = H + BS + 1  # packB: [d1w (H cols) | d2w_col xBS | d1b_col]

_BUILD_CACHE = {}


def _build_nc():
    nc = bacc.Bacc("TRN2", target_bir_lowering=False, debug=False)

    d_in = nc.dram_tensor("d_s", (BS, T - 1), F32, kind="ExternalInput")
    packC = nc.dram_tensor("packC", (BS, PC_COLS), F32, kind="ExternalInput")
    packB = nc.dram_tensor("packB", (H, PB_COLS), F32, kind="ExternalInput")
    out = nc.dram_tensor("out", (BS, 1), F32, kind="ExternalOutput")

    with tile.TileContext(nc) as tc:
        with (
            tc.tile_pool(name="sb", bufs=1) as sb,
            tc.tile_pool(name="ps", bufs=1, space=bass.MemorySpace.PSUM) as ps,
        ):
            pc = sb.tile([BS, PC_COLS], F32)
            pb = sb.tile([H, PB_COLS], F32)
            dcol = sb.tile([BS, 1], F32)
            # spread the three input loads over distinct DMA queues
            nc.sync.dma_start(pb[:, :], packB[:, :])
            nc.scalar.dma_start(pc[:, :], packC[:, :])
            nc.gpsimd.dma_start(dcol[:, :], d_in[:, T - 2:T - 1])

            onesr = sb.tile([1, BS], F32)
            nc.gpsimd.memset(onesr[:, :], 1.0)

            # din = d * dec_w00 + dec_b0            (BS,1)
            din = sb.tile([BS, 1], F32)
            nc.vector.tensor_scalar(
                din[:, :], dcol[:, :],
                pc[:, 6 * H:6 * H + 1], pc[:, 6 * H + 1:6 * H + 2],
                ALU.mult, ALU.add,
            )
            # z = Wrep * din + brep                 (BS, 3H), gates [i|o|g]
            z = sb.tile([BS, 3 * H], F32)
            nc.vector.scalar_tensor_tensor(
                z[:, :], pc[:, 0:3 * H], din[:, :], pc[:, 3 * H:6 * H],
                ALU.mult, ALU.add,
            )

            sio = sb.tile([BS, 2 * H], F32)  # sigmoid(i) | sigmoid(o)
            tg = sb.tile([BS, H], F32)
            nc.scalar.activation(sio[:, :], z[:, 0:2 * H], AF.Sigmoid)
            nc.scalar.activation(tg[:, :], z[:, 2 * H:3 * H], AF.Tanh)
            cst = sb.tile([BS, H], F32)
            nc.vector.tensor_mul(cst[:, :], sio[:, 0:H], tg[:, :])
            tcs = sb.tile([BS, H], F32)
            nc.scalar.activation(tcs[:, :], cst[:, :], AF.Tanh)
            hst = sb.tile([BS, H], F32)
            nc.vector.tensor_mul(hst[:, :], sio[:, H:2 * H], tcs[:, :])

            # vrep[b,h] = sum_j d2w[j] * d1w[j,h]   (BS, H)
            vrep = ps.tile([BS, H], F32)
            nc.tensor.matmul(
                vrep[:, :], pb[:, H:H + BS], pb[:, 0:H], start=True, stop=True
            )
            # krep[b] = sum_j d2w[j] * d1b[j] + d2b (BS, 1)
            krep = ps.tile([BS, 1], F32)
            nc.tensor.matmul(
                krep[:, :], pb[:, H:H + BS], pb[:, H + BS:H + BS + 1],
                start=True, stop=False,
            )
            nc.tensor.matmul(
                krep[:, :], onesr[:, :], pc[0:1, 6 * H + 2:6 * H + 3],
                start=False, stop=True,
            )

            # out[b] = sum_h h[b,h]*vrep[b,h] + krep[b]
            # (tensor_tensor_reduce crashes the exec unit on HW - use
            #  mul + reduce + scalar_add instead)
            scratch = sb.tile([BS, H], F32)
            res = sb.tile([BS, 1], F32)
            nc.vector.tensor_mul(scratch[:, :], hst[:, :], vrep[:, :])
            nc.vector.tensor_reduce(
                res[:, :], scratch[:, :], mybir.AxisListType.X, ALU.add
            )
            nc.vector.tensor_scalar_add(res[:, :], res[:, :], krep[:, 0:1])
            nc.sync.dma_start(out[:, :], res[:, :])

    nc.compile()
    return nc


def get_nc():
    if "nc" not in _BUILD_CACHE:
        _BUILD_CACHE["nc"] = _build_nc()
    return _BUILD_CACHE["nc"]


def make_in_maps(inputs):
    f = lambda k: np.asarray(inputs[k], dtype=np.float32)
    d = np.ascontiguousarray(f("d"))
    wihd = f("W_ih_d").reshape(4 * H)
    b_d = f("b_d").reshape(4 * H)
    dw = f("dec_w").reshape(1, H + 1)[0, 0]
    db = f("dec_b").reshape(1)[0]
    d1w = f("d1_w").reshape(H, 2 * H)
    d1b = f("d1_b").reshape(H)
    d2w = f("d2_w").reshape(H)
    d2b = f("d2_b").reshape(1)[0]

    # packC row (replicated per batch row): [W_i|W_o|W_g | b_i|b_o|b_g | dw db d2b]
    rowC = np.empty(PC_COLS, np.float32)
    rowC[0:H] = wihd[0:H]                  # i
    rowC[H:2 * H] = wihd[3 * H:4 * H]      # o
    rowC[2 * H:3 * H] = wihd[2 * H:3 * H]  # g
    rowC[3 * H:4 * H] = b_d[0:H]
    rowC[4 * H:5 * H] = b_d[3 * H:4 * H]
    rowC[5 * H:6 * H] = b_d[2 * H:3 * H]
    rowC[6 * H] = dw
    rowC[6 * H + 1] = db
    rowC[6 * H + 2] = d2b
    packC = np.ascontiguousarray(np.broadcast_to(rowC, (BS, PC_COLS)))

    # packB: [d1w[:, :H] | d2w_col xBS | d1b_col]
    packB = np.empty((H, PB_COLS), np.float32)
    packB[:, 0:H] = d1w[:, 0:H]
    packB[:, H:H + BS] = d2w[:, None]
    packB[:, H + BS] = d1b

    in_maps = []
    for c in range(N_CORES):
        in_maps.append(
            {
                "d_s": np.ascontiguousarray(d[c * BS:(c + 1) * BS]),
                "packC": packC,
                "packB": packB,
            }
        )
    return in_maps


def run_spmd(inputs, trace=False):
    """Returns (full_output (B,), BassKernelResults)."""
    nc = get_nc()
    res = bass_utils.run_bass_kernel_spmd(
        nc, make_in_maps(inputs), list(range(N_CORES)), trace=trace
    )
    outs = [np.asarray(res.results[c]["out"]).reshape(BS) for c in range(N_CORES)]
    full = np.concatenate(outs).astype(np.float32)
    return full, res


def kernel(**inputs) -> np.ndarray:
    full, _ = run_spmd(inputs, trace=False)
    return full


# revision 8
# speedup vs baseline: 1.2677x; 1.0765x over previous
"""Trainium2 Bass kernel for nn_DSA (dual-stage attention RNN).

Mathematical collapse used (exact, not approximate):
  - In the reference scan, beta = log_softmax(sc, axis=-1) over a SIZE-1
    axis, which is identically zero for any finite input.  Hence
    ctx_new = einsum('bt,bth->bh', 0, enc_h) == 0 exactly, so the carried
    context is zero at every step and the decoder input at step t is
    din_t = d[:, t] * dec_w[0,0] + dec_b[0].
  - The carried h_s is never read inside the step, so only the final
    step's h_s (t = T-2) reaches the head.  The encoder LSTM, s1, and the
    whole attention pipeline are dead code w.r.t. the output.
  - feat = [h_s, ctx] with ctx == 0, so the head reduces to
      out[b] = h_s[b,:] @ v + k0,
      v  = d1_w[:, :H].T @ d2_w[0,:],     k0 = d1_b @ d2_w[0,:] + d2_b[0]
  where h_s = sigmoid(o) * tanh(sigmoid(i) * tanh(g)) and
  [i,f,g,o] = din * W_ih_d[:,0] + b_d  (f unused since c0 == 0).

Sharding: pure data parallel over batch (B=32 -> 4 rows per core x 8).
All weights replicated; each core computes its 4 outputs independently.
Host-side work is layout only (slicing / replication / concatenation);
every arithmetic op ((d*dw+db), the LSTM cell, v, k0, h@v+k0) runs on
device.

Device schedule (per core, BS=4, batch on partitions):
  - TWO input DMAs on separate queues (sync HWDGE + gpsimd SWDGE):
      packM (BS, 776): [W_i|W_o|W_g | b_i|b_o|b_g | d_col dw db d2b 1x4]
      packB (H, 133):  [d1_w[:, :H] | d2w_col xBS | d1b_col]
  - DVE: din = d*dw+db; z = Wrep*din + brep (split io/g so the sigmoid
    starts earlier); ACT: one Sigmoid on (BS,256) covers both gates
    (no DMA on the Activation queue, so its function table loads once).
  - PE (off critical path): vrep = (d2w x4).T @ d1w; krep accumulates
    d1b.d2w + d2b via a ones-row matmul (ones baked into packM).
  - finale: mul + reduce + scalar-add (tensor_tensor_reduce faults the
    exec unit on HW; stick to plain DVE ops).
"""

import numpy as np

import concourse.bacc as bacc
import concourse.bass as bass
import concourse.mybir as mybir
import concourse.tile as tile
from concourse import bass_utils

N_CORES = 8
B, T, H, L = 32, 100, 128, 64
BS = B // N_CORES  # batch rows per core

F32 = mybir.dt.float32
AF = mybir.ActivationFunctionType
ALU = mybir.AluOpType

PM_COLS = 6 * H + 8   # [W(384) | b(384) | d dw db d2b | 1 1 1 1]
PB_COLS = H + BS + 1  # [d1w (H) | d2w_col xBS | d1b_col]

_BUILD_CACHE = {}


def _build_nc():
    nc = bacc.Bacc("TRN2", target_bir_lowering=False, debug=False)

    packM = nc.dram_tensor("packM", (BS, PM_COLS), F32, kind="ExternalInput")
    packB = nc.dram_tensor("packB", (H, PB_COLS), F32, kind="ExternalInput")
    out = nc.dram_tensor("out", (BS, 1), F32, kind="ExternalOutput")

    W0, B0, X0 = 0, 3 * H, 6 * H  # pack section offsets

    with tile.TileContext(nc) as tc:
        with (
            tc.tile_pool(name="sb", bufs=1) as sb,
            tc.tile_pool(name="ps", bufs=1, space=bass.MemorySpace.PSUM) as ps,
        ):
            pm = sb.tile([BS, PM_COLS], F32)
            pb = sb.tile([H, PB_COLS], F32)
            nc.sync.dma_start(pm[:, :], packM[:, :])
            nc.gpsimd.dma_start(pb[:, :], packB[:, :])

            # din = d * dec_w00 + dec_b0            (BS,1)
            din = sb.tile([BS, 1], F32)
            nc.vector.tensor_scalar(
                din[:, :], pm[:, X0:X0 + 1],
                pm[:, X0 + 1:X0 + 2], pm[:, X0 + 2:X0 + 3],
                ALU.mult, ALU.add,
            )
            # z = Wrep * din + brep, gates [i|o|g]; io first so ACT starts early
            z = sb.tile([BS, 3 * H], F32)
            nc.vector.scalar_tensor_tensor(
                z[:, 0:2 * H], pm[:, W0:W0 + 2 * H], din[:, :],
                pm[:, B0:B0 + 2 * H], ALU.mult, ALU.add,
            )
            nc.vector.scalar_tensor_tensor(
                z[:, 2 * H:3 * H], pm[:, W0 + 2 * H:W0 + 3 * H], din[:, :],
                pm[:, B0 + 2 * H:B0 + 3 * H], ALU.mult, ALU.add,
            )

            sio = sb.tile([BS, 2 * H], F32)  # sigmoid(i) | sigmoid(o)
            tg = sb.tile([BS, H], F32)
            nc.scalar.activation(sio[:, :], z[:, 0:2 * H], AF.Sigmoid)
            nc.scalar.activation(tg[:, :], z[:, 2 * H:3 * H], AF.Tanh)
            cst = sb.tile([BS, H], F32)
            nc.vector.tensor_mul(cst[:, :], sio[:, 0:H], tg[:, :])
            tcs = sb.tile([BS, H], F32)
            nc.scalar.activation(tcs[:, :], cst[:, :], AF.Tanh)
            hst = sb.tile([BS, H], F32)
            nc.vector.tensor_mul(hst[:, :], sio[:, H:2 * H], tcs[:, :])

            # vrep[b,h] = sum_j d2w[j] * d1w[j,h]   (BS, H)
            vrep = ps.tile([BS, H], F32)
            nc.tensor.matmul(
                vrep[:, :], pb[:, H:H + BS], pb[:, 0:H], start=True, stop=True
            )
            # krep[b] = sum_j d2w[j] * d1b[j] + d2b (BS, 1)
            krep = ps.tile([BS, 1], F32)
            nc.tensor.matmul(
                krep[:, :], pb[:, H:H + BS], pb[:, H + BS:H + BS + 1],
                start=True, stop=False,
            )
            nc.tensor.matmul(
                krep[:, :], pm[0:1, X0 + 4:X0 + 8], pm[0:1, X0 + 3:X0 + 4],
                start=False, stop=True,
            )

            # out[b] = sum_h h[b,h]*vrep[b,h] + krep[b]
            scratch = sb.tile([BS, H], F32)
            res = sb.tile([BS, 1], F32)
            nc.vector.tensor_mul(scratch[:, :], hst[:, :], vrep[:, :])
            nc.vector.tensor_reduce(
                res[:, :], scratch[:, :], mybir.AxisListType.X, ALU.add
            )
            nc.vector.tensor_scalar_add(res[:, :], res[:, :], krep[:, 0:1])
            nc.sync.dma_start(out[:, :], res[:, :])

    nc.compile()
    return nc


def get_nc():
    if "nc" not in _BUILD_CACHE:
        _BUILD_CACHE["nc"] = _build_nc()
    return _BUILD_CACHE["nc"]


def make_in_maps(inputs):
    f = lambda k: np.asarray(inputs[k], dtype=np.float32)
    d = f("d")
    wihd = f("W_ih_d").reshape(4 * H)
    b_d = f("b_d").reshape(4 * H)
    dw = f("dec_w").reshape(1, H + 1)[0, 0]
    db = f("dec_b").reshape(1)[0]
    d1w = f("d1_w").reshape(H, 2 * H)
    d1b = f("d1_b").reshape(H)
    d2w = f("d2_w").reshape(H)
    d2b = f("d2_b").reshape(1)[0]

    X0 = 6 * H
    base = np.empty(PM_COLS, np.float32)  # batch-independent part
    base[0:H] = wihd[0:H]                  # W_i
    base[H:2 * H] = wihd[3 * H:4 * H]      # W_o
    base[2 * H:3 * H] = wihd[2 * H:3 * H]  # W_g
    base[3 * H:4 * H] = b_d[0:H]
    base[4 * H:5 * H] = b_d[3 * H:4 * H]
    base[5 * H:6 * H] = b_d[2 * H:3 * H]
    base[X0 + 1] = dw
    base[X0 + 2] = db
    base[X0 + 3] = d2b
    base[X0 + 4:X0 + 8] = 1.0

    packB = np.empty((H, PB_COLS), np.float32)
    packB[:, 0:H] = d1w[:, 0:H]
    packB[:, H:H + BS] = d2w[:, None]
    packB[:, H + BS] = d1b

    in_maps = []
    for c in range(N_CORES):
        packM = np.tile(base, (BS, 1))
        packM[:, X0] = d[c * BS:(c + 1) * BS, T - 2]  # this core's d[:, T-2]
        in_maps.append({"packM": packM, "packB": packB})
    return in_maps


def run_spmd(inputs, trace=False):
    """Returns (full_output (B,), BassKernelResults)."""
    nc = get_nc()
    res = bass_utils.run_bass_kernel_spmd(
        nc, make_in_maps(inputs), list(range(N_CORES)), trace=trace
    )
    outs = [np.asarray(res.results[c]["out"]).reshape(BS) for c in range(N_CORES)]
    full = np.concatenate(outs).astype(np.float32)
    return full, res


def kernel(**inputs) -> np.ndarray:
    full, _ = run_spmd(inputs, trace=False)
    return full
